# revision 1
# baseline (speedup 1.0000x reference)
"""Trainium2 Bass kernel: NeptuneTransformerEncoderLayer on 8 NeuronCores, v2.

Sharding: batch(4) x seq-half(2) -> 8 cores, zero collectives (as baseline).
Each core: K/V for its batch's full 2048 tokens, Q/attention/FFN for its
own 1024 tokens (host permutes src so own tokens are rows [0:1024)).

v2 strategy: fp8(e4m3) DoubleRow matmuls (0.5 cycles/row, 256-deep
contraction per instruction):
  - parity-split rope: even/odd head dims are separate qkv matmuls, so
    rotated (even,odd) pairs land in free-dim slots -> scores run DoubleRow
    over the head dim.
  - PV flipped: probs stationary, V moving, out token-major [128q, 65]
    with a ones column giving the softmax denominator; normalization is a
    per-partition tensor_scalar_mul by the reciprocal of column 64.
  - FFN: w1/w3 stationary as fp8 hi+lo pairs (~8-bit mantissa), moving
    XN2T8 fp8; w2 flipped (stationary hidden, moving w2 fp8) so the output
    is token-major and the residual add needs no transposes.
All weight reformatting/quantization happens on the host.
"""
import sys

for _p in ("/opt/trn_rl_repo", "/root/.axon_site/_ro/trn_rl_repo"):
    if _p not in sys.path:
        sys.path.insert(0, _p)

import numpy as np

import concourse.bass as bass
import concourse.mybir as mybir
import concourse.tile as tile
from concourse import bacc
from concourse import bass_utils

F8 = mybir.dt.float8e4
F16 = mybir.dt.float16
F32 = mybir.dt.float32
AF = mybir.ActivationFunctionType
DR = mybir.MatmulPerfMode.DoubleRow
NP8 = mybir.dt.np(F8)

P = 128
D = 1024
NH = 16
HD = 64
DFF = 4096
FC = 32          # ff chunks of 128
S = 2048
TQ = 1024
HQ = 512         # query half
N_CORES = 8
EPS = 1e-5
BASE = 10000.0

# static pow2 scales (xavier bounds are shape-determined, so these are safe)
SX = 32.0        # x_norm1 -> f8
SQKW = 4096.0    # w_qkv quant
SQK = 32.0       # roped q/k -> f8
SVA = 32.0       # v -> f8 (becomes the attn-out scale after normalization)
SP_ = 2.0        # probs scale inside exp
SWO = 2048.0     # w_out quant
SX2 = 32.0       # x_norm2 -> f8
SW1 = 4096.0     # w1 quant
SH = 4.0         # hidden -> f8  (w3 carries the SH/SX2 fold)
SW3 = 4096.0     # w3 quant (before the SH/SX2 fold)
SW2 = 4096.0     # w2 quant


def build_nc():
    nc = bacc.Bacc("TRN2", target_bir_lowering=False, debug=False,
                   num_devices=N_CORES)
    T = {}
    T["src"] = nc.dram_tensor("src", [S, D], F32, kind="ExternalInput")
    T["cos_t"] = nc.dram_tensor("cos_t", [P, S], F16, kind="ExternalInput")
    T["sin_t"] = nc.dram_tensor("sin_t", [P, S], F16, kind="ExternalInput")
    # stationary q/k tiles: [qk, g, pi, p, m, i, r]
    T["wqk8"] = nc.dram_tensor("wqk8", [2, 4, 2, P, 4, 2, P], F8,
                               kind="ExternalInput")
    # moving v / out-proj / w2: [p, m, i, n]
    T["wv8"] = nc.dram_tensor("wv8", [P, 4, 2, D], F8, kind="ExternalInput")
    T["wo8"] = nc.dram_tensor("wo8", [P, 4, 2, D], F8, kind="ExternalInput")
    T["w2s8"] = nc.dram_tensor("w2s8", [P, 16, 2, D], F8, kind="ExternalInput")
    # stationary w1/w3 hi+lo: [fc, p, m, i, r]
    for nm in ("w1h8", "w1l8", "w3h8", "w3l8"):
        T[nm] = nc.dram_tensor(nm, [FC, P, 4, 2, P], F8, kind="ExternalInput")
    T["out"] = nc.dram_tensor("out", [TQ, D], F16, kind="ExternalOutput")

    with tile.TileContext(nc) as tc:
        emit(nc, tc, T)
    nc.compile()
    return nc


def emit(nc, tc, T):
    from contextlib import ExitStack
    ctx = ExitStack()
    with ctx:
        persist = ctx.enter_context(tc.tile_pool(name="persist", bufs=1))
        gA = ctx.enter_context(ExitStack())
        pA = gA.enter_context(tc.tile_pool(name="pA", bufs=1, side="right"))

        # persistent tiles
        eps_t = persist.tile([P, 1], F32)
        nc.vector.memset(eps_t[:], EPS)
        lnp_t = persist.tile([P, 1], F32)
        nc.vector.memset(lnp_t[:], float(np.log(SP_)))
        wo8s = persist.tile([P, 4, 2, D], F8)
        QT8 = persist.tile([P, 4, 2, TQ], F8)     # [32h'+jp, g, par, t]
        KT8 = persist.tile([P, 4, 2, S], F8)
        VA = persist.tile([P, 16, 16, 65], F8)    # [key, kc, h, vd|one]

        # phase-scoped (freed before the FFN weave peaks)
        C2 = pA.tile([P, S], F16)
        S2 = pA.tile([P, S], F16)
        XNT8 = pA.tile([P, 4, 2, S], F8)          # [dp, m, i, t]
        wv8s = pA.tile([P, 4, 2, D], F8)

        nc.sync.dma_start(C2[:], T["cos_t"][:])
        nc.sync.dma_start(S2[:], T["sin_t"][:])
        nc.sync.dma_start(wv8s[:], T["wv8"][:])
        nc.sync.dma_start(wo8s[:], T["wo8"][:])

        # ones columns of VA (v evacs don't touch column 64)
        nc.vector.memset(VA[:, :, :, 64], 1.0)

        src = T["src"]
        with tc.tile_pool(name="pha", bufs=3) as pha, \
             tc.tile_pool(name="pha_s", bufs=4) as pha_s, \
             tc.tile_pool(name="pwqk", bufs=1) as pwqk, \
             tc.tile_pool(name="prope", bufs=2) as prope, \
             tc.tile_pool(name="ps_qk", bufs=2, space="PSUM") as ps_qk, \
             tc.tile_pool(name="ps_v", bufs=2, space="PSUM") as ps_v:

            def rms_rinv(pool_s, ssq, tagp):
                # rms = exp(0.5*ln(ssq/D + eps)): stays in the exp table set
                lnv = pool_s.tile([P, 1], F32, tag=f"ln{tagp}", name="lnv")
                nc.scalar.activation(lnv[:], ssq[:], AF.Ln,
                                     bias=eps_t[:], scale=1.0 / D)
                rms = pool_s.tile([P, 1], F32, tag=f"rm{tagp}", name="rms")
                nc.scalar.activation(rms[:], lnv[:], AF.Exp, scale=0.5)
                rinv = pool_s.tile([P, 1], F32, tag=f"ri{tagp}", name="rinv")
                nc.vector.reciprocal(rinv[:], rms[:])
                return rinv

            def phase_a_load(ti):
                st = pha.tile([P, D], F32, tag="src_in", name="st")
                nc.sync.dma_start(st[:], src[ti * P:(ti + 1) * P, :])
                return st

            def phase_a(ti, st):
                scr = pha.tile([P, D], F32, tag="scr")
                ssq = pha_s.tile([P, 1], F32, tag="ssq")
                nc.scalar.activation(scr[:], st[:], AF.Square,
                                     accum_out=ssq[:])
                rinv = rms_rinv(pha_s, ssq, "1")
                xn = pha.tile([P, D], F16, tag="xn")
                nc.vector.tensor_scalar_mul(xn[:], st[:], rinv[:])
                xt = pha.tile([P, 8, P], F16, tag="xt")
                nc.scalar.dma_start(xt[:], xn[:], transpose=True)
                # cast f16 -> f8 with scale SX; chunks c=(2m+i) in order
                nc.vector.tensor_scalar_mul(
                    XNT8[:, :, :, ti * P:(ti + 1) * P],
                    xt[:].rearrange("p (m i) t -> p m i t", i=2), SX)

            def load_wqk(qk, g, pi):
                w = pwqk.tile([P, 4, 2, P], F8, tag=f"wqk{qk}{g}{pi}",
                              name=f"wqk{qk}{g}{pi}")
                nc.sync.dma_start(w[:], T["wqk8"].ap()[qk, g, pi])
                return w

            def mm_qk(dst8, qk, g, sl, wA, wB):
                """q or k for group g, token slice sl -> dst8[:, g, :, sl]"""
                L = sl.stop - sl.start
                pk = {}
                for pi in range(2):
                    pk[pi] = ps_qk.tile([P, HQ], F32, tag=f"pk{pi}",
                                        name=f"pk{pi}")
                    w = (wA, wB)[pi]
                    for m in range(4):
                        nc.tensor.matmul(pk[pi][:, 0:L], w[:, m, :, :],
                                         XNT8[:, m, :, sl],
                                         start=(m == 0), stop=(m == 3),
                                         perf_mode=DR)
                e = {}
                for pi in range(2):
                    e[pi] = prope.tile([P, HQ], F16, tag=f"e{pi}",
                                       name=f"e{pi}")
                    nc.scalar.activation(e[pi][:, 0:L], pk[pi][:, 0:L],
                                         AF.Copy, scale=1.0 / (SQKW * SX))
                t1 = prope.tile([P, HQ], F16, tag="t1")
                t2 = prope.tile([P, HQ], F16, tag="t2")
                nc.vector.tensor_mul(t1[:, 0:L], e[0][:, 0:L], C2[:, sl])
                nc.vector.tensor_mul(t2[:, 0:L], e[1][:, 0:L], S2[:, sl])
                re = prope.tile([P, HQ], F16, tag="re")
                nc.vector.tensor_sub(re[:, 0:L], t1[:, 0:L], t2[:, 0:L])
                nc.vector.tensor_scalar_mul(dst8[:, g, 0, sl], re[:, 0:L], SQK)
                nc.vector.tensor_mul(t1[:, 0:L], e[1][:, 0:L], C2[:, sl])
                nc.vector.tensor_mul(t2[:, 0:L], e[0][:, 0:L], S2[:, sl])
                ro = prope.tile([P, HQ], F16, tag="ro")
                nc.vector.tensor_add(ro[:, 0:L], t1[:, 0:L], t2[:, 0:L])
                nc.vector.tensor_scalar_mul(dst8[:, g, 1, sl], ro[:, 0:L], SQK)

            def mm_v(ti):
                pv = ps_v.tile([P, D], F32, tag="pv")
                tsl = slice(ti * P, (ti + 1) * P)
                for half in range(2):
                    dsl = slice(half * HQ, half * HQ + HQ)
                    for m in range(4):
                        nc.tensor.matmul(pv[:, dsl], XNT8[:, m, :, tsl],
                                         wv8s[:, m, :, dsl],
                                         start=(m == 0), stop=(m == 3),
                                         perf_mode=DR)
                nc.scalar.activation(
                    VA[:, ti, :, 0:64],
                    pv[:].rearrange("p (h v) -> p h v", v=64),
                    AF.Copy, scale=SVA / (SQKW * SX))

            # weave: phase A tiles, then q/k/v as their spans become ready
            sts = {}
            for ti in range(3):
                sts[ti] = phase_a_load(ti)
            def pa(ti):
                if ti + 3 < 16:
                    sts[ti + 3] = phase_a_load(ti + 3)
                phase_a(ti, sts.pop(ti))
            for ti in range(4):
                pa(ti)
            wq = {(g, pi): load_wqk(0, g, pi) for g in range(4)
                  for pi in range(2)}
            for ti in range(4, 8):
                pa(ti)
            for g in range(4):
                mm_qk(QT8, 0, g, slice(0, HQ), wq[(g, 0)], wq[(g, 1)])
            mm_v(0)
            mm_v(1)
            for ti in range(8, 12):
                pa(ti)
            for g in range(4):
                mm_qk(QT8, 0, g, slice(HQ, TQ), wq[(g, 0)], wq[(g, 1)])
            wk = {(g, pi): load_wqk(1, g, pi) for g in range(4)
                  for pi in range(2)}
            for ti in range(12, 16):
                pa(ti)
            for ks in range(2):
                for g in range(4):
                    mm_qk(KT8, 1, g, slice(ks * HQ, ks * HQ + HQ),
                          wk[(g, 0)], wk[(g, 1)])
            for ti in range(2, 9):
                mm_v(ti)
            for ks in range(2, 4):
                for g in range(4):
                    mm_qk(KT8, 1, g, slice(ks * HQ, ks * HQ + HQ),
                          wk[(g, 0)], wk[(g, 1)])
            for ti in range(9, 16):
                mm_v(ti)

        gA.close()   # free C2/S2/XNT8/wv8s

        # ---------------- era A: attention (+ transpose/out_proj woven) ----
        with tc.tile_pool(name="persist2", bufs=1) as persist2:
            w2s = persist2.tile([P, 16, 2, D], F8)
            nc.sync.dma_start(w2s[:], T["w2s8"][:])
            ATT = persist2.tile([P, 8, TQ], F16)      # [q, qb, 64h+dh]
            X2 = persist2.tile([P, 8, D], F16)        # [t, tb, d]
            XN2T8 = persist2.tile([P, 4, 2, TQ], F8)
            ATTT8 = persist2.tile([P, 4, 2, TQ], F8)
            HT8 = persist2.tile([P, FC, HQ], F8)      # per-half, reused

            def rms_rinv2(pool_s, ssq, tagp):
                lnv = pool_s.tile([P, 1], F32, tag=f"ln{tagp}", name="lnv")
                nc.scalar.activation(lnv[:], ssq[:], AF.Ln,
                                     bias=eps_t[:], scale=1.0 / D)
                rms = pool_s.tile([P, 1], F32, tag=f"rm{tagp}", name="rms")
                nc.scalar.activation(rms[:], lnv[:], AF.Exp, scale=0.5)
                rinv = pool_s.tile([P, 1], F32, tag=f"ri{tagp}", name="rinv")
                nc.vector.reciprocal(rinv[:], rms[:])
                return rinv

            with tc.tile_pool(name="ppt", bufs=1) as ppt, \
                 tc.tile_pool(name="patt", bufs=3) as patt, \
                 tc.tile_pool(name="pod", bufs=3) as pod, \
                 tc.tile_pool(name="pod_s", bufs=4) as pod_s, \
                 tc.tile_pool(name="ps_st", bufs=2, space="PSUM") as ps_st, \
                 tc.tile_pool(name="ps_pq", bufs=1, space="PSUM") as ps_pq, \
                 tc.tile_pool(name="ps_op", bufs=2, space="PSUM") as ps_op:

                def scores_exp(h, half, pt8):
                    g, hp = h // 4, h % 4
                    psl = slice(32 * hp, 32 * hp + 32)
                    qsl = slice(half * HQ, half * HQ + HQ)
                    for kcp in range(8):
                        pst = ps_st.tile([P, 2, HQ], F32, tag="pst")
                        for j in range(2):
                            kc = 2 * kcp + j
                            nc.tensor.matmul(
                                pst[:, j, :],
                                KT8[psl, g, :, kc * P:(kc + 1) * P],
                                QT8[psl, g, :, qsl],
                                start=True, stop=True, perf_mode=DR,
                                tile_position=(32 * hp, 0))
                        nc.scalar.activation(
                            pt8[:, 2 * kcp:2 * kcp + 2, :], pst[:],
                            AF.Exp, scale=0.125 / (SQK * SQK),
                            bias=lnp_t[:])

                def pv_head(h, half, pt8):
                    for qb in range(4):
                        pq2 = ps_pq.tile([P, 2, P], F32, tag="pq", name="pq2")
                        pq = pq2[:, (h * 4 + qb) % 2, :]
                        for kc in range(16):
                            nc.tensor.matmul(pq[:, 0:65],
                                             pt8[:, kc, qb * P:(qb + 1) * P],
                                             VA[:, kc, h, :],
                                             start=(kc == 0), stop=(kc == 15))
                        rec = patt.tile([P, 1], F32, tag="rec")
                        nc.vector.reciprocal(rec[:], pq[:, 64:65])
                        qbg = half * 4 + qb
                        nc.vector.tensor_scalar_mul(
                            ATT[:, qbg, 64 * h:64 * h + 64],
                            pq[:, 0:64], rec[:])

                def transpose_att(qb):
                    at = patt.tile([P, 8, P], F16, tag="at")
                    nc.scalar.dma_start(at[:], ATT[:, qb, :], transpose=True)
                    nc.vector.tensor_scalar_mul(
                        ATTT8[:, :, :, qb * P:(qb + 1) * P],
                        at[:].rearrange("p (m i) t -> p m i t", i=2), 1.0)

                def out_proj(tb):
                    tsl = slice(tb * P, (tb + 1) * P)
                    py16 = pod.tile([P, D], F16, tag="big1")
                    for piece in range(2):
                        dsl = slice(piece * HQ, piece * HQ + HQ)
                        pop = ps_op.tile([P, HQ], F32, tag="pop")
                        for m in range(4):
                            nc.tensor.matmul(pop[:], ATTT8[:, m, :, tsl],
                                             wo8s[:, m, :, dsl],
                                             start=(m == 0), stop=(m == 3),
                                             perf_mode=DR)
                        nc.vector.tensor_scalar_mul(py16[:, dsl], pop[:],
                                                    1.0 / (SVA * SWO))
                    srcq = pod.tile([P, D], F16, tag="big2")
                    nc.gpsimd.dma_start(srcq[:], src[tsl, :])
                    nc.vector.tensor_add(X2[:, tb, :], py16[:], srcq[:])
                    scr = pod.tile([P, D], F32, tag="sq32")
                    ssq = pod_s.tile([P, 1], F32, tag="ssq2")
                    nc.scalar.activation(scr[:], X2[:, tb, :], AF.Square,
                                         accum_out=ssq[:])
                    rinv = rms_rinv2(pod_s, ssq, "2")
                    xn2 = pod.tile([P, D], F16, tag="big3")
                    nc.vector.tensor_scalar_mul(xn2[:], X2[:, tb, :], rinv[:])
                    xt2 = pod.tile([P, 8, P], F16, tag="big2")
                    nc.scalar.dma_start(xt2[:], xn2[:], transpose=True)
                    nc.vector.tensor_scalar_mul(
                        XN2T8[:, :, :, tsl],
                        xt2[:].rearrange("p (m i) t -> p m i t", i=2), SX2)

                # half 0: pure attention
                pt = {}
                for h in range(16):
                    pt[h] = ppt.tile([P, 16, HQ], F8, tag=f"pt{h % 2}",
                                     name=f"pt{h % 2}")
                    scores_exp(h, 0, pt[h])
                    if h > 0:
                        pv_head(h - 1, 0, pt[h - 1])
                pv_head(15, 0, pt[15])

                # half 1 attention with half-0 transposes/out_proj woven
                weave = [[("tr", 0), ("tr", 1)], [("tr", 2), ("tr", 3)],
                         [("op", 0)], [("op", 1)], [("op", 2)], [("op", 3)]]
                slots = {3: 0, 5: 1, 7: 2, 9: 3, 11: 4, 13: 5}
                for h in range(16):
                    pt[h] = ppt.tile([P, 16, HQ], F8, tag=f"pt{h % 2}",
                                     name=f"pt{h % 2}")
                    scores_exp(h, 1, pt[h])
                    if h > 0:
                        pv_head(h - 1, 1, pt[h - 1])
                    if h in slots:
                        for kind, a in weave[slots[h]]:
                            if kind == "tr":
                                transpose_att(a)
                            else:
                                out_proj(a)
                pv_head(15, 1, pt[15])
                # half-1 epilogue: transposes + out_proj (tb 4..8)
                for qb in range(4, 8):
                    transpose_att(qb)
                for tb in range(4, 8):
                    out_proj(tb)

            # ---------------- era B: FFN, fully pipelined ----------------
            with tc.tile_pool(name="pffw", bufs=2) as pffw, \
                 tc.tile_pool(name="pff", bufs=3) as pff, \
                 tc.tile_pool(name="pod2", bufs=3) as pod2, \
                 tc.tile_pool(name="ps_ab", bufs=2, space="PSUM") as ps_ab, \
                 tc.tile_pool(name="ps_acc", bufs=4, space="PSUM") as ps_acc:

                def load_w13(q):
                    tiles = {}
                    for nm in ("w1h8", "w1l8", "w3h8", "w3l8"):
                        w = pffw.tile([P, 4, 4, 2, P], F8, tag=f"s{nm}",
                                      name=f"s{nm}")
                        nc.sync.dma_start(
                            w[:], T[nm].ap()[4 * q:4 * q + 4].rearrange(
                                "f p m i r -> p f m i r"))
                        tiles[nm] = w
                    return tiles

                def ffn_fc(fc, half, w13):
                    qsl = slice(half * HQ, half * HQ + HQ)
                    fq = fc % 4
                    pab = ps_ab.tile([P, 2, HQ], F32, tag="pab")
                    for j, wh, wl in ((0, "w1h8", "w1l8"),
                                      (1, "w3h8", "w3l8")):
                        for m in range(4):
                            nc.tensor.matmul(pab[:, j, :],
                                             w13[wh][:, fq, m, :, :],
                                             XN2T8[:, m, :, qsl],
                                             start=(m == 0), stop=False,
                                             perf_mode=DR)
                        for m in range(4):
                            nc.tensor.matmul(pab[:, j, :],
                                             w13[wl][:, fq, m, :, :],
                                             XN2T8[:, m, :, qsl],
                                             start=False, stop=(m == 3),
                                             perf_mode=DR)
                    sg = pff.tile([P, HQ], F16, tag="sg")
                    nc.scalar.activation(sg[:], pab[:, 0, :], AF.Sigmoid,
                                         scale=1.0 / (SX2 * SW1))
                    t16 = pff.tile([P, HQ], F16, tag="t16")
                    nc.vector.tensor_mul(t16[:], sg[:], pab[:, 1, :])
                    u32 = pff.tile([P, HQ], F32, tag="u32")
                    nc.vector.tensor_scalar_mul(
                        u32[:], t16[:], 16.0 / (SW3 * SX2 * SW1))
                    nc.vector.tensor_mul(HT8[:, fc, :], u32[:],
                                         pab[:, 0, :])

                def ffn_w2_tb(tb, half):
                    tg = half * 4 + tb
                    tsl = slice(tb * P, (tb + 1) * P)
                    of = pod2.tile([P, D], F16, tag="of")
                    for piece in range(2):
                        dsl = slice(piece * HQ, piece * HQ + HQ)
                        pacc = ps_acc.tile([P, HQ], F32, tag="pacc")
                        for m in range(16):
                            nc.tensor.matmul(
                                pacc[:], HT8[:, 2 * m:2 * m + 2, tsl],
                                w2s[:, m, :, dsl],
                                start=(m == 0), stop=(m == 15),
                                perf_mode=DR)
                        pz16 = pod2.tile([P, HQ], F16, tag="pz16")
                        nc.vector.tensor_scalar_mul(pz16[:], pacc[:],
                                                    1.0 / (SH * SW2))
                        nc.vector.tensor_add(of[:, dsl], pz16[:],
                                             X2[:, tg, dsl])
                    nc.gpsimd.dma_start(T["out"][tg * P:(tg + 1) * P, :],
                                        of[:])

                for half in range(2):
                    w13t = {0: load_w13(0)}
                    for fc in range(FC):
                        q = fc // 4
                        if fc % 4 == 0 and q < 7:
                            w13t[q + 1] = load_w13(q + 1)
                        ffn_fc(fc, half, w13t[q])
                    for tb in range(4):
                        ffn_w2_tb(tb, half)


_NC_CACHE = None


def _get_nc():
    global _NC_CACHE
    if _NC_CACHE is None:
        _NC_CACHE = build_nc()
    return _NC_CACHE


def _q8(x, scale):
    return np.clip(np.asarray(x, np.float32) * scale,
                   -240.0, 240.0).astype(NP8)


def _hilo(w, scale):
    hi = _q8(w, scale)
    resid = np.asarray(w, np.float32) * scale - hi.astype(np.float32)
    lo = np.clip(resid, -240.0, 240.0).astype(NP8)
    return hi, lo


def _host_tables(positions_b, axis_scale):
    """cos/sin [128, S] f16: row p = 32h'+jp -> table row jp (4x tiled)."""
    coord = positions_b * axis_scale[None, :]               # (S, 4)
    invf = BASE ** (-(np.arange(0, 16, 2, dtype=np.float32) / 16.0))  # (8,)
    ang = coord[:, :, None] * invf[None, None, :]           # (S, 4, 8)
    ang = ang.reshape(S, 32).T                              # (32, S): jp = 8a+j
    c = np.tile(np.cos(ang), (4, 1)).astype(np.float16)
    s = np.tile(np.sin(ang), (4, 1)).astype(np.float16)
    return c, s


def _prep_weights(inputs):
    n1 = np.asarray(inputs["norm1_w"], np.float32)
    n2 = np.asarray(inputs["norm2_w"], np.float32)
    w_qkv = np.asarray(inputs["w_qkv"], np.float32) * n1[None, :]
    w_out = np.asarray(inputs["w_out"], np.float32)
    w1 = np.asarray(inputs["w1"], np.float32) * n2[None, :]
    w3 = np.asarray(inputs["w3"], np.float32) * n2[None, :] * (SH / (SX2 * 16.0))
    w2 = np.asarray(inputs["w2"], np.float32)

    # wqk8 [2, 4, 2, p, m, i, r]: row = base + 64*(4g+h') + 2*jp + pi,
    # d = 128*(2m+i) + p ; r = 32h' + jp
    qk = np.stack([w_qkv[0:D], w_qkv[D:2 * D]], axis=0)
    qk = qk.reshape(2, 4, 4, 32, 2, 4, 2, P)   # [qk, g, h', jp, pi, m, i, p]
    wqk8 = _q8(qk.transpose(0, 1, 4, 7, 5, 6, 2, 3).reshape(
        2, 4, 2, P, 4, 2, 4 * 32), SQKW)

    # wv8 [p, m, i, n]: n = vd natural, d = 128*(2m+i)+p
    wv = w_qkv[2 * D:3 * D].T.reshape(4, 2, P, D)           # [m, i, p, vd]
    wv8 = _q8(wv.transpose(2, 0, 1, 3), SQKW)

    wo = w_out.T.reshape(4, 2, P, D)
    wo8 = _q8(wo.transpose(2, 0, 1, 3), SWO)

    def prep13(w, scale):
        t = w.reshape(FC, P, 4, 2, P)                       # [fc, r, m, i, p]
        t = t.transpose(0, 4, 2, 3, 1)                      # [fc, p, m, i, r]
        return _hilo(t, scale)

    w1h, w1l = prep13(w1, SW1)
    w3h, w3l = prep13(w3, SW3)

    t = w2.T.reshape(16, 2, P, D).transpose(2, 0, 1, 3)     # [p, m, i, dout]
    w2s8 = _q8(t, SW2)

    return dict(wqk8=wqk8, wv8=wv8, wo8=wo8, w1h8=w1h, w1l8=w1l,
                w3h8=w3h, w3l8=w3l, w2s8=w2s8)


def build_in_maps(inputs):
    src = np.asarray(inputs["src"], dtype=np.float32)
    positions = np.asarray(inputs["positions"], dtype=np.float32)
    axis_scale = np.asarray(inputs["axis_scale"], np.float32)
    weights = _prep_weights(inputs)
    in_maps = []
    for c in range(N_CORES):
        b, h = c // 2, c % 2
        sp = src[b]
        pp = positions[b]
        if h == 1:
            sp = np.concatenate([sp[TQ:], sp[:TQ]], axis=0)
            pp = np.concatenate([pp[TQ:], pp[:TQ]], axis=0)
        ct, st = _host_tables(pp, axis_scale)
        m = {"src": np.ascontiguousarray(sp), "cos_t": ct, "sin_t": st}
        m.update(weights)
        in_maps.append(m)
    return in_maps


def kernel(src, positions, w_qkv, w_out, norm1_w, norm2_w, w1, w2, w3,
           axis_scale):
    src = np.asarray(src, dtype=np.float32)
    B = src.shape[0]
    in_maps = build_in_maps(dict(
        src=src, positions=positions, w_qkv=w_qkv, w_out=w_out,
        norm1_w=norm1_w, norm2_w=norm2_w, w1=w1, w2=w2, w3=w3,
        axis_scale=axis_scale))
    nc = _get_nc()
    res = bass_utils.run_bass_kernel_spmd(nc, in_maps,
                                          core_ids=list(range(N_CORES)))
    outp = np.zeros((B, S, D), np.float32)
    for c in range(N_CORES):
        b, h = c // 2, c % 2
        outp[b, h * TQ:(h + 1) * TQ, :] = np.asarray(
            res.results[c]["out"], np.float32)
    return outp

